# revision 7
# baseline (speedup 1.0000x reference)
"""Trainium2 Bass kernel for classical self-attention (B=4, N=4096, D=1024, fp32).

  q = x @ Wq.T ; k = x @ Wk.T
  out = softmax(q @ k.T / sqrt(D)) @ x

Sharding: 8 cores = (batch b = c//2) x (query half h = c%2, 2048 queries each).
Each core holds all 4096 keys of its batch, so softmax rows are core-local and
no collectives are needed.

Device algorithm (per core); all matmuls run in float32r, which is full PE rate
at free-dim >= 256 (measured 228 ns per [128,128]x[128,512] MM incl. weight
load) and carries ~1e-4 relative error end-to-end vs the fp32 reference:

  phase 0: kT = Wk-projection of x into transposed layout [D, N] (the host
           supplies x.T and W.T so no on-chip transposes are needed), spilled
           to DRAM scratch per 512-key strip; qT strips stay resident in SBUF.
  per 512-query super-block:
    phase A: scores are computed directly TRANSPOSED: pT[n, q] chunks
             (contraction over the projected dim with kT slices as the
             stationary operand), exp on ScalarE straight PSUM->SBUF
             (scale=1/32 folded in; no max-subtraction: scores ~ N(0,1), so
             fp32 exp is exact-safe), and softmax denominators s[q] via
             ones-vector matmuls (a partition-dim reduction on the PE).
    phase B: out[q, :] accumulated in PSUM over all 32 key chunks with pT
             chunks as stationary operands and natural-layout x as the moving
             operand; normalization by 1/s happens once at PSUM eviction.

This file also carries two workarounds for this container's walrus build,
which rejects any instruction carrying more than one sync wait.
"""

import re

import numpy as np

import bass_rust
import concourse.bass as bass
import concourse.mybir as mybir
from concourse.tile import TileContext

B, N, D = 4, 4096, 1024
NQ = N // 2          # queries per core
QS = 512             # query super-block / key strip width
P = 128              # partitions
DC = D // P          # contraction chunks
F32 = mybir.dt.float32
F32R = mybir.dt.float32r
EXP = mybir.ActivationFunctionType.Exp
SCALE = 1.0 / 32.0   # 1/sqrt(D)
N_CORES = 8


class SplitDrainTileContext(TileContext):
    """The TileContext exit emits one SP Drain waiting on every proc's final
    semaphore value; this walrus build allows a single sync wait per
    instruction.  Emit the waits as single-wait NOPs first, then a drain
    that needs no waits of its own."""

    def _drain_and_barrier(self, tick_clock, wait_clock):
        gc = tick_clock.global_clock
        ticks = [int(s) for s in re.findall(r"\d+", repr(gc))]
        for proc, t in enumerate(ticks):
            if t > 0:
                single = bass_rust.VectorClock()
                single.require_at_least(proc, t)
                nop = self.nc.sync.nop(nofuse=True, hint="split_drain_wait")
                wait_clock.add_sem_waits(nop.ins, bass_rust.ScopedClock({None: single}))
        drain_inst = self.nc.sync.drain()
        wait_clock.add_sem_waits(
            drain_inst.ins,
            bass_rust.ScopedClock({None: gc}),
            bass_rust.ScopedClock({None: gc.copy()}),
        )
        self.nc.all_engine_barrier()
        assert self.sems is not None
        popped = self.nc._tile_sem_poison_stack.pop()
        assert popped is self._sem_poison
        self.nc.clear_and_free_semaphores(list(self.sems.allocated().values()))
        self.nc.all_engine_barrier()


def _split_multiwaits(nc: bass.Bass, max_waits: int = 1) -> None:
    """Hoist extra sync waits onto injected NoOps placed immediately before
    the instruction in the same basic block (engines execute their stream in
    bb order, so the engine blocks on each NoOp's wait before reaching the
    real instruction)."""
    ctr = 0
    for bb in nc.main_func.blocks:
        new_list = []
        changed = False
        for inst in bb.instructions:
            si = inst.sync_info
            if si is not None and len(si.on_wait) > max_waits:
                waits = list(si.on_wait)
                keep = waits[-max_waits:]
                for w in waits[:-max_waits]:
                    nop = mybir.InstNoOp(name=f"splitw-{ctr}", ins=[], outs=[])
                    ctr += 1
                    nop.engine = inst.engine
                    nop.sync_info = mybir.SyncInfo(on_wait=[w], on_update=[])
                    new_list.append(nop)
                inst.sync_info = mybir.SyncInfo(
                    on_wait=keep, on_update=list(si.on_update)
                )
                changed = True
            new_list.append(inst)
        if changed:
            bb.instructions = new_list


def build_kernel() -> bass.Bass:
    nc = bass.Bass()
    x_nat = nc.dram_tensor("x", [N, D], F32R, kind="ExternalInput")
    xt = nc.dram_tensor("xt", [D, N], F32R, kind="ExternalInput")
    xtq = nc.dram_tensor("xtq", [D, NQ], F32R, kind="ExternalInput")
    wqt = nc.dram_tensor("wqt", [D, D], F32R, kind="ExternalInput")
    wkt = nc.dram_tensor("wkt", [D, D], F32R, kind="ExternalInput")
    out = nc.dram_tensor("out", [NQ, D], F32, kind="ExternalOutput")

    xt_r = xt.rearrange("(c p) n -> p c n", p=P)
    xtq_r = xtq.rearrange("(c p) n -> p c n", p=P)
    wqt_r = wqt.rearrange("(c p) e -> p c e", p=P)
    wkt_r = wkt.rearrange("(c p) e -> p c e", p=P)

    NQS = NQ // QS        # 4 query super-blocks
    NCH = N // P          # 32 key chunks
    NSTRIP = N // QS      # 8 key strips

    with SplitDrainTileContext(nc) as tc:
        with (
            tc.tile_pool(name="dram", bufs=1, space="DRAM") as dpool,
            tc.tile_pool(name="psum", bufs=8, space="PSUM") as pp,
            tc.tile_pool(name="persist", bufs=1) as persist,
            tc.tile_pool(name="qtp", bufs=1) as qtp,
        ):
            # per-strip kT scratch so phase-A reads depend only on their strip
            kt_ds = [
                dpool.tile([D, QS], F32R, name=f"kt_d{i}", tag=f"kt_d{i}")
                for i in range(NSTRIP)
            ]
            s_d = dpool.tile([NQS, QS], F32, name="s_d", tag="s_d")

            ones_f32 = persist.tile([P, 1], F32, name="ones_f32", tag="ones32")
            nc.vector.memset(ones_f32, 1.0)
            ones_t = persist.tile([P, 1], F32R, name="ones_t", tag="ones")
            nc.scalar.copy(ones_t, ones_f32)

            # qt strips stay resident in SBUF for the whole kernel (8MB)
            qt_strips = [
                qtp.tile([P, DC, QS], F32R, name=f"qt_strip{i}", tag=f"qt{i}")
                for i in range(NQS)
            ]

            # ---------------- phase 0: projections ------------------------
            with (
                tc.tile_pool(name="wpool", bufs=1) as wpool,
                tc.tile_pool(name="p0x", bufs=3) as p0x,
                tc.tile_pool(name="p0c", bufs=4) as p0c,
            ):
                wq_sb = wpool.tile([P, DC, D], F32R, name="wq_sb", tag="wq")
                for h in range(4):
                    nc.sync.dma_start(
                        out=wq_sb[:, h * DC // 4:(h + 1) * DC // 4, :],
                        in_=wqt_r[:, h * DC // 4:(h + 1) * DC // 4, :],
                    )
                wk_sb = wpool.tile([P, DC, D], F32R, name="wk_sb", tag="wk")
                for h in range(4):
                    nc.sync.dma_start(
                        out=wk_sb[:, h * DC // 4:(h + 1) * DC // 4, :],
                        in_=wkt_r[:, h * DC // 4:(h + 1) * DC // 4, :],
                    )

                def project_q(qb):
                    xblk = p0x.tile([P, DC, QS], F32R, name="xblk", tag="xblk")
                    nc.sync.dma_start(
                        out=xblk, in_=xtq_r[:, :, qb * QS:(qb + 1) * QS]
                    )
                    for e in range(DC):
                        ps = pp.tile([P, QS], F32, name="ps0", tag="bank")
                        for d in range(DC):
                            nc.tensor.matmul(
                                ps,
                                wq_sb[:, d, e * P:(e + 1) * P],
                                xblk[:, d, :],
                                start=(d == 0),
                                stop=(d == DC - 1),
                            )
                        nc.scalar.copy(qt_strips[qb][:, e, :], ps)

                def project_k(nb):
                    xblk = p0x.tile([P, DC, QS], F32R, name="xblk", tag="xblk")
                    nc.sync.dma_start(
                        out=xblk, in_=xt_r[:, :, nb * QS:(nb + 1) * QS]
                    )
                    for e in range(DC):
                        ps = pp.tile([P, QS], F32, name="ps0", tag="bank")
                        for d in range(DC):
                            nc.tensor.matmul(
                                ps,
                                wk_sb[:, d, e * P:(e + 1) * P],
                                xblk[:, d, :],
                                start=(d == 0),
                                stop=(d == DC - 1),
                            )
                        cp = p0c.tile([P, QS], F32R, name="cp", tag="cp")
                        nc.scalar.copy(cp, ps)
                        nc.sync.dma_start(
                            out=kt_ds[nb][e * P:(e + 1) * P, :],
                            in_=cp,
                        )

                # q-block 0 first: it gates the first score matmuls
                project_q(0)
                for nb in range(NSTRIP):
                    project_k(nb)
                for qb in range(1, NQS):
                    project_q(qb)

            # ---------------- main loop ----------------------------------
            with (
                tc.tile_pool(name="ktp", bufs=3) as ktp,
                tc.tile_pool(name="ptp", bufs=1) as ptp,
                tc.tile_pool(name="xbp", bufs=4) as xbp,
                tc.tile_pool(name="outp", bufs=4) as outp,
                tc.tile_pool(name="smallp", bufs=2) as smallp,
            ):
                for qs in range(NQS):
                    q0 = qs * QS
                    qt_strip = qt_strips[qs]

                    # phase A: pT chunks + row-sums
                    pt_tiles = []
                    ps_s = pp.tile([1, QS], F32, name="ps_s", tag="bank")
                    for ns in range(NSTRIP):
                        kt_strip = ktp.tile(
                            [P, DC, QS], F32R, name="kt_strip", tag="kt"
                        )
                        nc.sync.dma_start(
                            out=kt_strip,
                            in_=kt_ds[ns].rearrange("(c p) n -> p c n", p=P),
                        )
                        for j in range(QS // P):
                            nk = ns * (QS // P) + j
                            ps = pp.tile([P, QS], F32, name="ps_sc", tag="bank")
                            for e in range(DC):
                                nc.tensor.matmul(
                                    ps,
                                    kt_strip[:, e, j * P:(j + 1) * P],
                                    qt_strip[:, e, :],
                                    start=(e == 0),
                                    stop=(e == DC - 1),
                                )
                            pt = ptp.tile([P, QS], F32R, name="pt", tag=f"pt{nk}")
                            nc.scalar.activation(pt, ps, EXP, scale=SCALE)
                            pt_tiles.append(pt)
                            nc.tensor.matmul(
                                ps_s,
                                ones_t,
                                pt,
                                start=(nk == 0),
                                stop=(nk == NCH - 1),
                            )

                    # denominators -> [128, 4] via a tiny DRAM roundtrip
                    s_sb = smallp.tile([1, QS], F32, name="s_sb", tag="s_sb")
                    nc.scalar.copy(s_sb, ps_s)
                    nc.sync.dma_start(out=s_d[qs:qs + 1, :], in_=s_sb)
                    s_resh = smallp.tile([P, QS // P], F32, name="s_resh", tag="s_resh")
                    nc.sync.dma_start(
                        out=s_resh,
                        in_=s_d.rearrange("r (a p) -> r p a", p=P)[qs],
                    )
                    recip = smallp.tile([P, QS // P], F32, name="recip", tag="recip")
                    nc.vector.reciprocal(recip, s_resh)

                    # phase B: out accumulation over all key chunks
                    ps_o = [
                        pp.tile([P, QS], F32, name="ps_o", tag="bank")
                        for _ in range(8)
                    ]
                    for nk in range(NCH):
                        xc = xbp.tile([P, D], F32R, name="xc", tag="xc")
                        nc.sync.dma_start(out=xc, in_=x_nat[nk * P:(nk + 1) * P, :])
                        for qsub in range(QS // P):
                            lhsT = pt_tiles[nk][:, qsub * P:(qsub + 1) * P]
                            for eh in range(2):
                                nc.tensor.matmul(
                                    ps_o[qsub * 2 + eh],
                                    lhsT,
                                    xc[:, eh * QS:(eh + 1) * QS],
                                    start=(nk == 0),
                                    stop=(nk == NCH - 1),
                                )
                    for qsub in range(QS // P):
                        for eh in range(2):
                            o_sb = outp.tile([P, QS], F32, name="o_sb", tag="o_sb")
                            nc.vector.tensor_scalar_mul(
                                o_sb, ps_o[qsub * 2 + eh], recip[:, qsub:qsub + 1]
                            )
                            nc.sync.dma_start(
                                out=out[
                                    q0 + qsub * P:q0 + (qsub + 1) * P,
                                    eh * QS:(eh + 1) * QS,
                                ],
                                in_=o_sb,
                            )
    _split_multiwaits(nc)
    return nc


def _make_in_maps(x, Wq, Wk):
    x = np.ascontiguousarray(x, dtype=np.float32)
    wqt = np.ascontiguousarray(np.asarray(Wq, dtype=np.float32).T)
    wkt = np.ascontiguousarray(np.asarray(Wk, dtype=np.float32).T)
    in_maps = []
    for c in range(N_CORES):
        b, h = divmod(c, 2)
        xtb = np.ascontiguousarray(x[b].T)
        in_maps.append(
            {
                "x": np.ascontiguousarray(x[b]),
                "xt": xtb,
                "xtq": np.ascontiguousarray(xtb[:, h * NQ:(h + 1) * NQ]),
                "wqt": wqt,
                "wkt": wkt,
            }
        )
    return in_maps


_NC_CACHE = None
_RUNNER_CACHE = None


def _make_runner(nc):
    """Build the sharded PJRT callable once so repeated kernel() calls reuse
    the jit cache (mirrors concourse.bass2jax.run_bass_via_pjrt's multi-core
    branch)."""
    import jax
    from jax.experimental.shard_map import shard_map
    from jax.sharding import Mesh, PartitionSpec

    from concourse import bass2jax

    bass2jax.install_neuronx_cc_hook()

    partition_name = nc.partition_id_tensor.name if nc.partition_id_tensor else None
    in_names, out_names, out_avals, zero_outs = [], [], [], []
    for alloc in nc.m.functions[0].allocations:
        if not isinstance(alloc, mybir.MemoryLocationSet):
            continue
        name = alloc.memorylocations[0].name
        if alloc.kind == "ExternalInput":
            if name != partition_name:
                in_names.append(name)
        elif alloc.kind == "ExternalOutput":
            shape = tuple(alloc.tensor_shape)
            dtype = mybir.dt.np(alloc.dtype)
            out_names.append(name)
            out_avals.append(jax.core.ShapedArray(shape, dtype))
            zero_outs.append(np.zeros(shape, dtype))
    n_params = len(in_names)
    n_outs = len(out_avals)
    all_in_names = list(in_names) + list(out_names)
    if partition_name is not None:
        all_in_names.append(partition_name)
    donate = tuple(range(n_params, n_params + n_outs))

    def _body(*args):
        operands = list(args)
        if partition_name is not None:
            operands.append(bass2jax.partition_id_tensor())
        outs = bass2jax._bass_exec_p.bind(
            *operands,
            out_avals=tuple(out_avals),
            in_names=tuple(all_in_names),
            out_names=tuple(out_names),
            lowering_input_output_aliases=(),
            sim_require_finite=True,
            sim_require_nnan=True,
            nc=nc,
        )
        return tuple(outs)

    devices = jax.devices()[:N_CORES]
    mesh = Mesh(np.asarray(devices), ("core",))
    in_specs = (PartitionSpec("core"),) * (n_params + n_outs)
    out_specs = (PartitionSpec("core"),) * n_outs
    sharded = jax.jit(
        shard_map(
            _body, mesh=mesh, in_specs=in_specs, out_specs=out_specs,
            check_rep=False,
        ),
        donate_argnums=donate,
        keep_unused=True,
    )

    def run(in_maps):
        concat_in = [
            np.concatenate([np.asarray(m[nm]) for m in in_maps], axis=0)
            for nm in in_names
        ]
        concat_zeros = [
            np.zeros((N_CORES * z.shape[0], *z.shape[1:]), z.dtype)
            for z in zero_outs
        ]
        out_arrs = sharded(*concat_in, *concat_zeros)
        return [
            {
                nm: np.asarray(out_arrs[i]).reshape(
                    N_CORES, *out_avals[i].shape
                )[c]
                for i, nm in enumerate(out_names)
            }
            for c in range(N_CORES)
        ]

    return run


def kernel(x: np.ndarray, Wq: np.ndarray, Wk: np.ndarray) -> np.ndarray:
    global _NC_CACHE, _RUNNER_CACHE
    if _NC_CACHE is None:
        _NC_CACHE = build_kernel()
    nc = _NC_CACHE

    in_maps = _make_in_maps(x, Wq, Wk)

    results = None
    try:
        if _RUNNER_CACHE is None:
            _RUNNER_CACHE = _make_runner(nc)
        results = _RUNNER_CACHE(in_maps)
    except Exception:
        _RUNNER_CACHE = None
        results = None
    if results is None:
        # fallback: the supported (slower, per-call jit) path
        from concourse.bass_utils import run_bass_kernel_spmd

        results = run_bass_kernel_spmd(
            nc, in_maps, core_ids=list(range(N_CORES))
        ).results

    outv = np.empty((B, N, D), dtype=np.float32)
    for c in range(N_CORES):
        b, h = divmod(c, 2)
        outv[b, h * NQ:(h + 1) * NQ, :] = results[c]["out"]
    return outv
